# revision 2
# baseline (speedup 1.0000x reference)
"""Trainium2 Bass kernel for nn_Attention_86646670230179 (eager MHA, f32 I/O).

v2 strategy (8 NeuronCores, DP2 x TP4, collective-free):
  - Core c owns batch c//4 and heads 4*(c%4)..4*(c%4)+4 (256-channel slice).
  - Inputs staged n-major ([128, NT, KT, 512] bf16) so staging is 12 x 1MB
    chunks ordered k0,v0,q0,k1,v1,...: attention pass 0 starts ~9us in,
    chasing the DMA.
  - q/k projected in transposed layout (channels on partitions); v projected
    in NATURAL layout (keys on partitions) by swapping matmul operands
    (lhsT = x^T chunk, rhs = Wv), eliminating on-chip transposes.
  - Per (lt, head-pair) pass: 16 key-tiles; scores^T via two half-array
    matmuls (tile_position row packing), exp on ScalarE ([128,1024] per
    key-tile, the critical resource), PV accumulation with an appended
    ones-column giving row sums for free.
  - Normalization deferred one pass (PSUM->SBUF copy, gpsimd broadcast of
    the sums row straight from PSUM, reciprocal_approx_fast, multiply).
  - Out-projection per lt (both head-pairs accumulated in one PSUM group),
    written to DRAM as bf16 via gpsimd casting DMA directly from PSUM.
  - Host sums the 4 per-core partials per batch (the TP all-reduce) and adds
    (bv @ Wo + bo), which commutes with attention since softmax rows sum to 1.
"""
import sys
from contextlib import ExitStack

import numpy as np

sys.path.insert(0, "/opt/trn_rl_repo")

import ml_dtypes  # noqa: E402
import concourse.bass as bass  # noqa: E402
import concourse.mybir as mybir  # noqa: E402
import concourse.tile as tile  # noqa: E402
from concourse import bacc  # noqa: E402
from concourse.bass_utils import run_bass_kernel_spmd  # noqa: E402

BF16 = mybir.dt.bfloat16
F32 = mybir.dt.float32
AF = mybir.ActivationFunctionType

NCORES = 8
B, L, E, H = 2, 2048, 1024, 16
S = L
D = E // H            # 64 head dim
DP = 2                # data-parallel groups (batch)
TP = NCORES // DP     # 4 tensor-parallel cores per batch
HC = H // TP          # 4 heads per core
EC = HC * D           # 256 channel slice per core
HP = HC // 2          # 2 head-pairs per core
KT = E // 128         # 8 contraction tiles
NT = L // 512         # 4 512-row blocks
ST = S // 128         # 16 key tiles
STN = ST // NT        # 4 key tiles per 512-row block
DP1 = D + 1           # 65: head dim + ones column


def build_nc():
    nc = bacc.Bacc("TRN2", target_bir_lowering=False, num_devices=NCORES)

    xq = nc.declare_dram_parameter("xq", [128, NT, KT, 512], BF16, isOutput=False)
    xk = nc.declare_dram_parameter("xk", [128, NT, KT, 512], BF16, isOutput=False)
    xv = nc.declare_dram_parameter("xv", [128, NT, KT, 512], BF16, isOutput=False)
    wq = nc.declare_dram_parameter("wq", [128, KT * EC], BF16, isOutput=False)
    wk = nc.declare_dram_parameter("wk", [128, KT * EC], BF16, isOutput=False)
    wv = nc.declare_dram_parameter("wv", [128, KT * EC], BF16, isOutput=False)
    wo = nc.declare_dram_parameter("wo", [128, HP * E], BF16, isOutput=False)
    bq = nc.declare_dram_parameter("bq", [128, HP], F32, isOutput=False)
    bk = nc.declare_dram_parameter("bk", [128, HP], F32, isOutput=False)
    outT = nc.declare_dram_parameter("outT", [E, L], BF16, isOutput=True)

    with tile.TileContext(nc) as tc, ExitStack() as ctx:
        consts = ctx.enter_context(tc.tile_pool(name="consts", bufs=1))
        ex_pool = ctx.enter_context(tc.tile_pool(name="expp", bufs=4))
        pou_pool = ctx.enter_context(tc.tile_pool(name="poup", bufs=4))
        rci_pool = ctx.enter_context(tc.tile_pool(name="rcip", bufs=6))
        rcb_pool = ctx.enter_context(tc.tile_pool(name="rcbp", bufs=2))
        ot_pool = ctx.enter_context(tc.tile_pool(name="otp", bufs=3))
        ov_pool = ctx.enter_context(tc.tile_pool(name="ovp", bufs=8))
        # PSUM banks: sc 2x[128,1024] (4) + pv 2x[128,512] (2) + pp 2 (2) = 8
        psum_sc = ctx.enter_context(tc.tile_pool(name="psc", bufs=2, space="PSUM"))
        psum_pv = ctx.enter_context(tc.tile_pool(name="ppv", bufs=2, space="PSUM"))
        psum_pp = ctx.enter_context(tc.tile_pool(name="ppp", bufs=2, space="PSUM"))

        # ---- weights / biases
        wq_sb = consts.tile([128, KT, EC], BF16, tag="wq")
        wk_sb = consts.tile([128, KT, EC], BF16, tag="wk")
        wv_sb = consts.tile([128, KT, EC], BF16, tag="wv")
        wo_sb = consts.tile([128, HP, E], BF16, tag="wo")
        bq_sb = consts.tile([128, HP], F32, tag="bq")
        bk_sb = consts.tile([128, HP], F32, tag="bk")
        nc.gpsimd.dma_start(bq_sb[:], bq[:])
        nc.gpsimd.dma_start(bk_sb[:], bk[:])

        # ---- activation staging tiles (one per tensor per n-block)
        xt = {}
        for name in ("k", "v", "q"):
            for n in range(NT):
                xt[(name, n)] = consts.tile(
                    [128, KT, 512], BF16, tag=f"xt{name}{n}", name=f"xt{name}{n}"
                )
        # DMA priority order: the chunk queue is strictly serial (~2.8us/MB),
        # so exactly what act0 needs goes first: wk, k0, wq, q0.
        nc.sync.dma_start(wk_sb[:], wk[:].rearrange("p (ko m) -> p ko m", m=EC))
        src = {"k": xk, "v": xv, "q": xq}
        nc.sync.dma_start(xt[("k", 0)][:], xk[:, 0])
        nc.sync.dma_start(wq_sb[:], wq[:].rearrange("p (ko m) -> p ko m", m=EC))
        nc.sync.dma_start(xt[("q", 0)][:], xq[:, 0])
        nc.sync.dma_start(wv_sb[:], wv[:].rearrange("p (ko m) -> p ko m", m=EC))
        nc.sync.dma_start(xt[("v", 0)][:], xv[:, 0])
        chunk_order = [("k", 1), ("v", 1), ("q", 1), ("k", 2), ("v", 2)]
        for name, n in chunk_order:
            nc.sync.dma_start(xt[(name, n)][:], src[name][:, n])
        nc.sync.dma_start(wo_sb[:], wo[:].rearrange("p (h m) -> p h m", m=E))
        for name, n in (("k", 3), ("v", 3), ("q", 2), ("q", 3)):
            nc.sync.dma_start(xt[(name, n)][:], src[name][:, n])

        # ---- projection outputs
        qpT = [[consts.tile([128, 512], BF16, tag=f"qpT{hp}_{n}",
                            name=f"qpT{hp}_{n}") for n in range(NT)]
               for hp in range(HP)]
        kpT = [[consts.tile([128, 512], BF16, tag=f"kpT{hp}_{n}",
                            name=f"kpT{hp}_{n}") for n in range(NT)]
               for hp in range(HP)]
        vp = [consts.tile([128, STN, HC, DP1], BF16, tag=f"vp{n}", name=f"vp{n}")
              for n in range(NT)]
        for n in range(NT):
            nc.vector.memset(vp[n][:, :, :, D], 1.0)

        def proj_qk(name, n, only_hp=None):
            """Transposed-layout projection of q or k for one n-block."""
            w_sb, bias, dsts = {
                "k": (wk_sb, bk_sb, kpT),
                "q": (wq_sb, bq_sb, qpT),
            }[name]
            xts = xt[(name, n)]
            for hp in (range(HP) if only_hp is None else (only_hp,)):
                ps = psum_pp.tile([128, 512], F32, tag="pp")
                for kt in range(KT):
                    nc.tensor.matmul(
                        ps[:],
                        lhsT=w_sb[:, kt, hp * 128:(hp + 1) * 128],
                        rhs=xts[:, kt, :],
                        start=(kt == 0),
                        stop=(kt == KT - 1),
                    )
                nc.vector.tensor_tensor(
                    dsts[hp][n][:], ps[:],
                    bias[:, hp:hp + 1].to_broadcast((128, 512)),
                    mybir.AluOpType.add,
                )

        def proj_v(n, half=None):
            """Natural-layout projection of v (keys on partitions)."""
            xts = xt[("v", n)]
            mbs = range(STN) if half is None else range(2 * half, 2 * half + 2)
            for mb in mbs:
                ps = psum_pp.tile([128, 512], F32, tag="pp")
                for kt in range(KT):
                    nc.tensor.matmul(
                        ps[:, 0:EC],
                        lhsT=xts[:, kt, mb * 128:(mb + 1) * 128],
                        rhs=wv_sb[:, kt, :],
                        start=(kt == 0),
                        stop=(kt == KT - 1),
                    )
                nc.vector.tensor_copy(
                    vp[n][:, mb, :, 0:D],
                    ps[:, 0:EC].rearrange("p (h d) -> p h d", d=D),
                )

        # deferred work queues
        pending_norm = []   # [(pou, rci, lt, hp, h01), ...]
        pending_mul = []    # [(pou, rcb, lt, hp, h01), ...]
        pending_proj = []   # (lt, m) out-projection groups
        ots = {}            # (lt, hp) -> ot tile

        def norm_copies(po, lt, hp):
            """Emitted right after a pass's last PV: pull PV data + sums out
            of PSUM (frees po banks), and compute 1/sums.

            In the tail (last pass) the po banks are never recycled, so the
            data copy is skipped (the mul reads PSUM directly) and the sums
            copies split across ScalarE (idle by then) and DVE."""
            tail = lt == NT - 1 and hp == HP - 1
            for h01 in range(2):
                if tail:
                    pou_ap = po[h01][0:D, :]
                else:
                    pou = pou_pool.tile([D, 512], BF16, tag="pou")
                    nc.vector.tensor_copy(pou[:], po[h01][0:D, :])
                    pou_ap = pou[:]
                sm = rci_pool.tile([1, 512], F32, tag="sm")
                if tail and h01 == 1:
                    nc.scalar.copy(sm[:], po[h01][D:DP1, :])
                else:
                    nc.vector.tensor_copy(sm[:], po[h01][D:DP1, :])
                rci = rci_pool.tile([1, 512], F32, tag="rci")
                nc.vector.reciprocal_approx_fast(rci[:], sm[:])
                pending_norm.append((pou_ap, rci, lt, hp, h01))
            # broadcast right away (gpsimd is idle); muls run next pass
            norm_bcast()

        def norm_bcast():
            """Broadcast 1/sums for both heads of the oldest pending pass."""
            for pou_ap, rci, flt, fhp, h01 in pending_norm[:2]:
                rcb = rcb_pool.tile([D, 512], F32, tag="rcb")
                nc.gpsimd.partition_broadcast(rcb[:], rci[:])
                pending_mul.append((pou_ap, rcb, flt, fhp, h01))
            del pending_norm[:2]

        def norm_mul():
            pou_ap, rcb, flt, fhp, h01 = pending_mul.pop(0)
            if h01 == 0:
                ots[(flt, fhp)] = ot_pool.tile(
                    [128, 512], BF16, tag="ot", name=f"ot{flt}_{fhp}"
                )
            ot = ots[(flt, fhp)]
            nc.vector.tensor_tensor(
                ot[h01 * D:(h01 + 1) * D, :], pou_ap, rcb[:],
                mybir.AluOpType.mult,
            )
            if h01 == 1 and fhp == 1:
                for m in range(KT):
                    pending_proj.append((flt, m))

        def proj_one(tail=False):
            flt, m = pending_proj.pop(0)
            # in the tail, rotate PSUM through the idle sc pool too (4-deep)
            pool = psum_sc if tail and m % 2 else psum_pp
            pt = pool.tile([128, 512], F32, tag="pp" if pool is psum_pp else "sc")
            for hp in range(HP):
                nc.tensor.matmul(
                    pt[:],
                    lhsT=wo_sb[:, hp, m * 128:(m + 1) * 128],
                    rhs=ots[(flt, hp)][:],
                    start=(hp == 0),
                    stop=(hp == HP - 1),
                )
            ov = ov_pool.tile([128, 512], BF16, tag="ov")
            if tail and m % 2:
                nc.scalar.copy(ov[:], pt[:])  # ScalarE is idle in the tail
            else:
                nc.vector.tensor_copy(ov[:], pt[:])
            dst = outT[m * 128:(m + 1) * 128, flt * 512:(flt + 1) * 512]
            if tail and m % 2:
                nc.gpsimd.dma_start(dst, ov[:])
            else:
                nc.sync.dma_start(dst, ov[:])

        # software-pipelined PV: emitted one st behind its scores/exp so the
        # next pass's scores never sit behind an act-dependent PV in the
        # in-order PE queue.
        pv_delay = []

        def flush_pv():
            while pv_delay:
                po, stn, mb, hp, st, lt, ex = pv_delay.pop(0)
                for h01 in range(2):
                    nc.tensor.matmul(
                        po[h01][0:DP1, :],
                        lhsT=vp[stn][:, mb, hp * 2 + h01, :],
                        rhs=ex[:, h01 * 512:(h01 + 1) * 512],
                        start=(st == 0),
                        stop=(st == ST - 1),
                    )
                if st == ST - 1:
                    norm_copies(po, lt, hp)

        def attention(lt, hp, extra=()):
            """One pass: 512 queries x all 2048 keys for one head-pair.
            extra[st] = thunk emitted after st's matmuls (projection chase)."""
            po = [psum_pv.tile([128, 512], F32, tag="pv", name=f"po{h}")
                  for h in range(2)]
            emitted_proj = [0]
            for st in range(ST):
                stn, mb = st // STN, st % STN
                ps = psum_sc.tile([128, 1024], F32, tag="sc")
                for h01 in range(2):
                    nc.tensor.matmul(
                        ps[:, h01 * 512:(h01 + 1) * 512],
                        lhsT=kpT[hp][stn][h01 * D:(h01 + 1) * D,
                                          mb * 128:(mb + 1) * 128],
                        rhs=qpT[hp][lt][h01 * D:(h01 + 1) * D, :],
                        start=True,
                        stop=True,
                        tile_position=(h01 * D, 0),
                    )
                ex = ex_pool.tile([128, 1024], BF16, tag="exp")
                nc.scalar.activation(ex[:], ps[:], AF.Exp)
                flush_pv()
                pv_delay.append((po, stn, mb, hp, st, lt, ex))
                # interleave deferred work into the ScalarE-gated slots
                if st in (0, 1) and pending_mul:
                    norm_mul()
                elif st >= 6 and pending_proj and emitted_proj[0] < 4:
                    proj_one()
                    emitted_proj[0] += 1
                if st in extra:
                    extra[st]()

        # ---- emission: projections chase the DMA; passes chase projections.
        # k/q first so scores(st0) fires ASAP; v and later-n fillers spread
        # across the pass-0 slots at <=2 PSUM groups per slot.
        proj_qk("k", 0)
        proj_qk("q", 0)
        first_extra = {
            0: lambda: proj_v(0, 0),
            1: lambda: proj_v(0, 1),
            3: lambda: proj_qk("k", 1),
            4: lambda: proj_v(1, 0),
            5: lambda: proj_v(1, 1),
            7: lambda: proj_qk("k", 2),
            8: lambda: proj_v(2, 0),
            9: lambda: proj_v(2, 1),
            11: lambda: proj_qk("k", 3),
            12: lambda: proj_v(3, 0),
            13: lambda: proj_v(3, 1),
        }
        for lt in range(NT):
            for hp in range(HP):
                extra = {}
                if lt == 0 and hp == 0:
                    extra = first_extra
                elif hp == 1 and lt < NT - 1:
                    extra = {6: (lambda n=lt + 1: proj_qk("q", n))}
                attention(lt, hp, extra)
        # tail flush
        flush_pv()
        # keep the PE in its high p-state through the norm chain: harmless
        # dummy matmuls into a dead PSUM bank (result never read)
        warm = psum_sc.tile([128, 1024], F32, tag="sc", name="warm")
        for _ in range(12):
            nc.tensor.matmul(
                warm[:, 0:512],
                lhsT=kpT[0][0][0:D, 0:128],
                rhs=qpT[0][0][0:D, :],
                start=True, stop=True,
                tile_position=(0, 0),
            )
        while pending_norm:
            norm_bcast()
        while pending_mul:
            norm_mul()
        while pending_proj:
            proj_one(tail=True)

    nc.compile()
    return nc


_NC_CACHE = {}


def _get_nc():
    if "nc" not in _NC_CACHE:
        _NC_CACHE["nc"] = build_nc()
    return _NC_CACHE["nc"]


def kernel(q, k, v, Wq, bq, Wk, bk, Wv, bv, Wo, bo, _trace=False, _tmpdir=None):
    bf = ml_dtypes.bfloat16
    scale = np.float32(1.0 / np.sqrt(D))  # 0.125, exact

    def _stage_x(x, b):
        # [L, E] -> [128, NT, KT, 512] n-major chunks
        xb = np.asarray(x[b], np.float32).reshape(NT, 512, KT, 128)
        return np.ascontiguousarray(xb.transpose(3, 0, 2, 1)).astype(bf)

    def _stage_w(w):
        # [E, EC] -> [128, KT*EC] partition-major
        return np.ascontiguousarray(
            w.reshape(KT, 128, EC).transpose(1, 0, 2).reshape(128, KT * EC)
        ).astype(bf)

    Wq = np.asarray(Wq, np.float32)
    Wk = np.asarray(Wk, np.float32)
    Wv = np.asarray(Wv, np.float32)
    Wo = np.asarray(Wo, np.float32)

    xs = {}
    for b in range(B):
        xs[b] = (_stage_x(q, b), _stage_x(k, b), _stage_x(v, b))

    in_maps = []
    for c in range(NCORES):
        b = c // TP
        t = c % TP
        sl = slice(t * EC, (t + 1) * EC)
        xqh, xkh, xvh = xs[b]
        wo_sl = Wo[sl, :].reshape(HP, 128, E).transpose(1, 0, 2)
        in_maps.append({
            "xq": xqh,
            "xk": xkh,
            "xv": xvh,
            "wq": _stage_w(Wq[:, sl] * scale),
            "wk": _stage_w(Wk[:, sl]),
            "wv": _stage_w(Wv[:, sl]),
            "wo": np.ascontiguousarray(wo_sl.reshape(128, HP * E)).astype(bf),
            "bq": np.ascontiguousarray(
                (np.asarray(bq, np.float32)[sl] * scale).reshape(HP, 128).T),
            "bk": np.ascontiguousarray(
                np.asarray(bk, np.float32)[sl].reshape(HP, 128).T),
        })

    nc = _get_nc()
    res = run_bass_kernel_spmd(
        nc, in_maps, list(range(NCORES)), trace=_trace, tmpdir=_tmpdir
    )
    # sum the per-core partial outputs (the TP all-reduce), per batch group
    out = np.empty((B, L, E), np.float32)
    for b in range(B):
        acc = np.zeros((E, L), np.float32)
        for t in range(TP):
            acc += np.asarray(res.results[b * TP + t]["outT"], np.float32)
        out[b] = acc.T
    host_bias = (
        np.asarray(bv, np.float64) @ np.asarray(Wo, np.float64)
        + np.asarray(bo, np.float64)
    ).astype(np.float32)
    out += host_bias[None, None, :]
    if _trace:
        return out, res
    return out
